# revision 21
# baseline (speedup 1.0000x reference)
"""Causal self-attention Trainium2 kernel (v6).

Full-model shapes: x [4, 2048, 1024], w_qkv [1024, 3072], b_qkv [3072],
w_out [1024, 1024], b_out [1024].  H=16 heads, D=64.

Sharding: 8 cores = 4 batches x 2 head-groups (tensor parallel).  Each core
computes qkv projection for its 8 heads of its batch, causal attention, and
the partial out-projection (512 of 1024 contraction rows).  The two partials
per batch are summed on the host (the "all-reduce" after out_proj), plus
b_out.

Evolution (traced on HW):
  v2 ~642us: 64 ACT_TABLE_LOADs (exp<->ln set thrash, 82us of ScalarE) +
     4-8us PE gaps at every (head, window) boundary kept PE_HAM throttled
     at 1.2 GHz for 413us (matmuls avg 436ns vs 216ns warm).
  v3 ~529us: activation-table map filtered to natural_log_exp_and_others
     (exp AND ln -> ONE table load); emission restructured for PE density
     (dependency-free qkv/out-proj m-tiles as filler between the
     exp-dependent PVs); 1/Z chain decoupled from PSUM via an SBUF copy.
  v4 ~336us: the v3 trace showed GpSimd UNLOAD_LIB/LOAD_LIB pairs (~6us)
     between partition_broadcast and the diag-mask tensor_mul every window
     -> all diagonal masks moved to the DVE additive path; GpSimd now runs
     ONLY partition_broadcast, one library, no swaps.  HAM stays warm
     (throttle 413us -> 25us).
  v5: q/k/v projections in fp8 DoubleRow: 315us but rel_err 3.5e-2 — numpy
     decomposition shows the v path dominates (y_t for small t is v_t with
     no softmax averaging; max-abs metric is set by those rows), q/k-only
     fp8 is 1.4e-2, out-proj fp8 3.6e-2.
  v6: v path and out-proj back to bf16; q/k stay fp8 DoubleRow (softmax
     soaks their quantization: errors cancel where few keys participate).
     x is streamed in both fp8 (q/k moving operand) and bf16 (v stationary
     operand).  Prefix computes only heads 0/1's q,k so the first window's
     exp starts ~25us earlier; everything else becomes need-scheduled
     filler dripped between s-blocks.

Layout per core:
  - xT [C, T] host-transposed, streamed as [128, KT, chunk] tiles (fp8 for
    q/k, bf16 for v).
  - qT [feat, T] (feature-on-partition, 2 heads per 128-tile), kz per-head
    K-padded [128, T] (64 rows k_h + 64 zero rows -> S matmuls contract over
    full 128 partitions).  q,k carry a x64 factor (fp8 weight scaling);
    the exp scale divides it back out.
  - v natural [T, feat] with a ones column per head so PV yields y_un and
    the softmax denominator Z in one PSUM accumulation.
  - scores S^T [s, t] per (s-block, t-half window); diagonal blocks masked
    pre-exp additive on the DVE.
"""

import sys
from contextlib import ExitStack

import numpy as np

sys.path.insert(0, "/opt/trn_rl_repo")

import ml_dtypes

import concourse.bacc as bacc
import concourse.bass as bass
import concourse.tile as tile
from concourse import mybir
from concourse.bass_utils import run_bass_kernel_spmd

F32 = mybir.dt.float32
BF16 = mybir.dt.bfloat16
BF = ml_dtypes.bfloat16
F8 = mybir.dt.float8e4
F8NP = ml_dtypes.float8_e4m3
DR = mybir.MatmulPerfMode.DoubleRow

B, T, C, H = 4, 2048, 1024, 16
D = C // H  # 64
N_CORES = 8
HL = H // 2      # heads per core = 8
FL = HL * D      # local features = 512
KT = C // 128    # 8 contraction tiles


# Pin every activation to the one table set that holds both exp and ln so
# the table-load fixpoint pass emits a single ACT_TABLE_LOAD.  v2 measured
# 64 loads (82us of ScalarE blockage + the PE stalls they cascade into).
_PIN_SET = "natural_log_exp_and_others"
_real_get_tables = bacc.get_activation_tables


def _pinned_get_tables(arch):
    real = _real_get_tables(arch)
    return {name: (fns if name == _PIN_SET else set())
            for name, fns in real.items()}


bacc.get_activation_tables = _pinned_get_tables


QK_FP8 = True         # q/k projection in fp8 DoubleRow (v, out-proj: bf16)
W_SCALE = 64.0        # fp8 q/k weights are scaled x64 into e4m3's band
EXP_W = 1024          # exp window width (1024 = cross-bank ACT reads)
LAG = 6               # deferred PV chunks: hides exp latency from the PE


def _chunks(t0, tend, grid=512):
    """Aligned chunks [c0, c0+w) covering [t0, tend), clipped to the global
    `grid` so no chunk crosses a grid (= PSUM bank) boundary."""
    out = []
    while t0 < tend:
        w = min(grid - (t0 % grid), tend - t0)
        out.append((t0, w))
        t0 += w
    return out


def build_program(t_len=T):
    nc = bacc.Bacc(None, target_bir_lowering=False, debug=False)
    TT = t_len
    n_tt = TT // 128
    QK_DT, QKNP = (F8, F8NP) if QK_FP8 else (BF16, BF)

    xT8 = nc.declare_dram_parameter("xT8", [C, TT], QK_DT, isOutput=False)
    xTb = nc.declare_dram_parameter("xTb", [C, TT], BF16, isOutput=False)
    # host-packed: wqk[p, m, k*128+f] = w_m[k*128+p, f]; m 0-3 q, 4-7 k tiles
    wqk = nc.declare_dram_parameter("wqk", [128, 8, KT * 128], QK_DT,
                                    isOutput=False)
    wv = nc.declare_dram_parameter("wv", [128, KT * FL], BF16, isOutput=False)
    wout = nc.declare_dram_parameter("wout", [FL, C], BF16, isOutput=False)
    bqk = nc.declare_dram_parameter("bqk", [128, 8], F32, isOutput=False)
    bv = nc.declare_dram_parameter("bv", [128, HL // 2], F32, isOutput=False)
    trineg = nc.declare_dram_parameter("trineg", [128, 128], F32,
                                       isOutput=False)
    out = nc.declare_dram_parameter("out", [TT, C], F32, isOutput=True)

    HW = min(1024, TT)   # t-half width for exp windows / S psum tiles
    QW = min(512, TT)    # y accumulation quarter width
    n_half = TT // HW

    with tile.TileContext(nc) as tc, ExitStack() as top:
        persist = top.enter_context(tc.tile_pool(name="persist", bufs=1))
        stream = top.enter_context(tc.tile_pool(name="stream", bufs=3))
        upool = top.enter_context(tc.tile_pool(name="u", bufs=LAG + 2))
        zpool = top.enter_context(tc.tile_pool(name="z", bufs=2))
        ycpool = top.enter_context(tc.tile_pool(name="yc", bufs=4))
        obpool = top.enter_context(tc.tile_pool(name="ob", bufs=3))

        wqk_sb = [persist.tile([128, KT * 128], QK_DT, tag=f"wqk{m}",
                               name=f"wqk{m}") for m in range(8)]
        wv_sb = persist.tile([128, KT * FL], BF16, tag="wv", name="wv_sb")
        qT_sb = [persist.tile([128, TT], BF16, tag=f"qT{j}", name=f"qT{j}")
                 for j in range(4)]
        kz_sb = [persist.tile([128, TT], BF16, tag=f"kz{h}", name=f"kz{h}")
                 for h in range(HL)]
        v_sb = persist.tile([128, n_tt, HL, D + 1], BF16, tag="v", name="v_sb")
        yT_sb = [persist.tile([128, TT], BF16, tag=f"yT{j}", name=f"yT{j}")
                 for j in range(4)]
        wout_sb = [persist.tile([128, C], BF16, tag=f"wo{j}", name=f"wo{j}")
                   for j in range(4)]
        bqk_sb = persist.tile([128, 8], F32, tag="bqk", name="bqk_sb")
        bv_sb = persist.tile([128, HL // 2], F32, tag="bv", name="bv_sb")
        trineg_sb = persist.tile([128, 128], F32, tag="trineg",
                                 name="trineg_sb")

        # -------- qkv projection over t-chunks; x streamed once -----------
        if TT >= 1024:
            achunks = [(0, 256), (256, 256)] + [
                (c, 512) for c in range(512, TT, 512)]
        else:
            achunks = [(c, 256) for c in range(0, TT, 256)]
        n_pre = len([c for c, _ in achunks if c < HW])  # chunks with t < HW
        xtiles8, xtilesb = {}, {}

        def load_chunk(ci):
            c0, ach = achunks[ci]
            xt8 = stream.tile([128, KT, 512], QK_DT, tag="x8", name=f"x8_{ci}")
            xtb = stream.tile([128, KT, 512], BF16, tag="xb", name=f"xb_{ci}")
            for k in range(KT):
                nc.sync.dma_start(
                    out=xt8[:, k, :ach],
                    in_=xT8.rearrange("(k p) t -> p k t", p=128)[:, k,
                                                                c0:c0 + ach])
                nc.sync.dma_start(
                    out=xtb[:, k, :ach],
                    in_=xTb.rearrange("(k p) t -> p k t", p=128)[:, k,
                                                                c0:c0 + ach])
            xtiles8[ci] = xt8
            xtilesb[ci] = xtb

        # first matmul needs wqk tile 0 + chunk 0: emit those DMAs first
        nc.sync.dma_start(out=wqk_sb[0], in_=wqk[:, 0, :])
        load_chunk(0)
        nc.sync.dma_start(out=wqk_sb[4], in_=wqk[:, 4, :])
        for m in (1, 2, 3, 5, 6, 7):
            nc.sync.dma_start(out=wqk_sb[m], in_=wqk[:, m, :])
        nc.sync.dma_start(out=wv_sb, in_=wv[:])
        nc.sync.dma_start(out=bqk_sb, in_=bqk[:])
        nc.sync.dma_start(out=bv_sb, in_=bv[:])
        nc.sync.dma_start(out=trineg_sb, in_=trineg[:])
        for h in range(HL):  # kz zero halves (the K-padding)
            zo = 64 * ((h + 1) % 2)
            nc.vector.memset(kz_sb[h][zo:zo + 64, :], 0.0)
        nc.vector.memset(v_sb[:, :, :, D], 1.0)  # PV ones column -> Z

        def emit_qk_tile(pool, ci, m):
            c0, ach = achunks[ci]
            xt = xtiles8[ci]
            ps = pool.tile([128, 512], F32, tag="mm", name="aps")
            if QK_FP8:  # DoubleRow: K=256 per matmul
                wm = wqk_sb[m].rearrange("p (kp i f) -> p kp i f", i=2, f=128)
                for kp in range(KT // 2):
                    nc.tensor.matmul(
                        ps[:, :ach],
                        wm[:, kp],
                        xt[:, 2 * kp:2 * kp + 2, :ach],
                        start=(kp == 0), stop=(kp == KT // 2 - 1),
                        perf_mode=DR)
            else:
                for k in range(KT):
                    nc.tensor.matmul(
                        ps[:, :ach],
                        wqk_sb[m][:, k * 128:(k + 1) * 128],
                        xt[:, k, :ach],
                        start=(k == 0), stop=(k == KT - 1))
            bias = bqk_sb[:, m:m + 1]
            if m < 4:
                nc.vector.tensor_scalar_add(
                    qT_sb[m][:, c0:c0 + ach], ps[:, :ach], bias)
            else:
                # split k across the two per-head K-padded tiles
                for par in range(2):
                    nc.vector.tensor_scalar_add(
                        kz_sb[(m - 4) * 2 + par][64 * par:64 * par + 64,
                                                 c0:c0 + ach],
                        ps[64 * par:64 * par + 64, :ach],
                        bias[64 * par:64 * par + 64, :])

        def emit_v_tile(pool, ci, sub):
            c0, ach = achunks[ci]
            xt = xtilesb[ci]
            ps = pool.tile([128, 512], F32, tag="mm", name="aps")
            for k in range(KT):
                nc.tensor.matmul(
                    ps[:, :FL],
                    xt[:, k, sub * 128:(sub + 1) * 128],
                    wv_sb[:, k * FL:(k + 1) * FL],
                    start=(k == 0), stop=(k == KT - 1))
            it = c0 // 128 + sub
            nc.vector.tensor_copy(
                out=v_sb[:, it, :, 0:D],
                in_=ps[:, :FL].rearrange("p (h d) -> p h d", h=HL))

        # prefix: ONLY heads 0/1's q,k over the t<HW chunks, so the first
        # attention window (and with it the ScalarE exp stream) starts as
        # early as possible.  Everything else drips in as filler.
        actx = ExitStack()
        a_pool = actx.enter_context(
            tc.tile_pool(name="a_psum", bufs=4, space="PSUM"))
        for ci in range(n_pre):
            if ci + 1 < len(achunks):
                load_chunk(ci + 1)
            emit_qk_tile(a_pool, ci, 0)
            emit_qk_tile(a_pool, ci, 4)
        actx.close()
        for j in range(4):  # w_out arrives during early attention
            nc.sync.dma_start(out=wout_sb[j], in_=wout[j * 128:(j + 1) * 128, :])

        # -------- attention; remaining qkv + out-proj interleaved ---------
        bctx = ExitStack()
        s_pool = bctx.enter_context(
            tc.tile_pool(name="s_psum", bufs=2, space="PSUM"))
        y_pool = bctx.enter_context(
            tc.tile_pool(name="y_psum", bufs=2, space="PSUM"))
        aux_pool = bctx.enter_context(
            tc.tile_pool(name="aux_psum", bufs=2, space="PSUM"))

        # filler units: dependency-free PE work threaded between the
        # exp-dependent PV matmuls to keep the PE queue dense (HAM warm).
        # Order: v for t<HW (feeds the first windows' PVs), remaining q/k
        # head pairs, then the t>=HW chunks, then out-proj m-tiles.
        fillers = []
        n_consumed = {"n": 0}
        for ci in range(n_pre):
            for sub in range(achunks[ci][1] // 128):
                fillers.append(
                    lambda ci=ci, sub=sub: emit_v_tile(aux_pool, ci, sub))
        n_v01 = len(fillers)
        for ci in range(n_pre):  # chunk-major: frees x stream slots fastest
            for mq in (1, 2, 3):
                fillers.append(lambda ci=ci, m=mq: emit_qk_tile(aux_pool, ci, m))
                fillers.append(
                    lambda ci=ci, m=mq + 4: emit_qk_tile(aux_pool, ci, m))
        for ci in range(n_pre, len(achunks)):
            load_chunk(ci)
            for m in range(8):
                fillers.append(lambda ci=ci, m=m: emit_qk_tile(aux_pool, ci, m))
            for sub in range(achunks[ci][1] // 128):
                fillers.append(
                    lambda ci=ci, sub=sub: emit_v_tile(aux_pool, ci, sub))
        n_a_units = len(fillers) + n_consumed["n"]

        def drain_fillers(n):
            """Emit filler units until at least n have been consumed."""
            while n_consumed["n"] < n and fillers:
                fillers.pop(0)()
                n_consumed["n"] += 1

        c_state = {"m": 0}

        def emit_c_tile():
            m = c_state["m"]
            c_state["m"] += 1
            ob = obpool.tile([128, C], F32, tag="ob", name="ob")
            for n in range(C // 512):
                ps = aux_pool.tile([128, 512], F32, tag="mm", name="cps")
                for j in range(4):
                    nc.tensor.matmul(
                        ps,
                        yT_sb[j][:, m * 128:(m + 1) * 128],
                        wout_sb[j][:, n * 512:(n + 1) * 512],
                        start=(j == 0), stop=(j == 3))
                nc.vector.tensor_copy(out=ob[:, n * 512:(n + 1) * 512], in_=ps)
            nc.sync.dma_start(out=out[m * 128:(m + 1) * 128, :], in_=ob)

        # deferred DVE normalize ops (window w's y*(1/Z)+b), drained at the
        # top of window w+1 so a mul waiting on the GpSimd broadcast never
        # blocks the strict-FIFO DVE queue in front of w+1's mask adds.
        norm_pending = []

        def window(h, half, drip):
            jt = h // 2           # q tile / yT tile index
            po = 64 * (h % 2)     # partition offset within the tile pair
            h0 = half * HW
            while norm_pending:
                norm_pending.pop(0)()
            yq = [y_pool.tile([D + 1, QW], F32, tag="y", name=f"yq{q}")
                  for q in range(HW // QW)]
            pend = []

            def emit_pv(s, c0, w, u):
                q = (c0 - h0) // QW
                nc.tensor.matmul(
                    yq[q][:, (c0 - h0) % QW:(c0 - h0) % QW + w],
                    v_sb[:, s, h, :],
                    u[:, c0 - h0:c0 - h0 + w],
                    start=(s == 0), stop=False,
                    skip_group_check=True)

            for s in range(n_tt):
                if s * 128 >= h0 + HW:
                    break
                t0 = max(s * 128, h0)
                loc0 = t0 - h0
                sps = s_pool.tile([128, HW], F32, tag="s", name="sps")
                chunk_list = _chunks(t0, h0 + HW)
                for (c0, w) in chunk_list:
                    nc.tensor.matmul(
                        sps[:, c0 - h0:c0 - h0 + w],
                        kz_sb[h][:, s * 128:(s + 1) * 128],
                        qT_sb[jt][:, c0:c0 + w],
                        start=True, stop=True,
                        skip_group_check=True)
                if s * 128 >= h0:  # pre-exp additive diag mask (DVE)
                    nc.vector.tensor_add(
                        sps[:, loc0:loc0 + 128], sps[:, loc0:loc0 + 128],
                        trineg_sb)
                u = upool.tile([128, HW], BF16, tag="u", name="u")
                for w0 in range(loc0 // EXP_W * EXP_W, HW, EXP_W):
                    a0, a1 = max(loc0, w0), min(w0 + EXP_W, HW)
                    nc.scalar.activation(
                        out=u[:, a0:a1], in_=sps[:, a0:a1],
                        func=mybir.ActivationFunctionType.Exp,
                        scale=1.0 / (np.sqrt(D) * W_SCALE * W_SCALE))
                for (c0, w) in chunk_list:
                    pend.append((s, c0, w, u))
                while len(pend) > LAG:
                    emit_pv(*pend.pop(0))
                if drip and (drip > 1 or s % 2 == 1):
                    drain_fillers(n_consumed["n"] + drip)
            for p in pend:
                emit_pv(*p)

            # deferred softmax normalization, decoupled from PSUM: one DVE
            # copy to SBUF bf16 frees the yq bank; 1/Z = exp(-ln Z) on the
            # Scalar engine (single pinned table set), GpSimd broadcast
            # from partition 0, bf16 DVE mul+bias into yT (deferred).
            for q in range(HW // QW):
                col0 = h0 + q * QW
                yc = ycpool.tile([D + 1, QW], BF16, tag="yc", name="yc")
                nc.vector.tensor_copy(out=yc, in_=yq[q])
                lnz = zpool.tile([1, QW], F32, tag="lnz", name="lnz")
                nc.scalar.activation(out=lnz, in_=yc[D:D + 1, :],
                                     func=mybir.ActivationFunctionType.Ln)
                rz = zpool.tile([1, QW], BF16, tag="rz", name="rz")
                nc.scalar.activation(out=rz, in_=lnz, scale=-1.0,
                                     func=mybir.ActivationFunctionType.Exp)
                rzb = zpool.tile([64, QW], BF16, tag="rzb", name="rzb")
                nc.gpsimd.partition_broadcast(rzb, rz)

                def norm(jt=jt, po=po, col0=col0, yc=yc, rzb=rzb):
                    dst = yT_sb[jt][po:po + 64, col0:col0 + QW]
                    nc.vector.tensor_mul(dst, yc[0:D, :], rzb)
                    nc.vector.tensor_scalar_add(
                        dst, dst, bv_sb[po:po + 64, jt:jt + 1])
                norm_pending.append(norm)

        for half in range(n_half):
            if half == n_half - 1:
                # out-proj m-tiles of completed halves become the filler;
                # all qkv units must be emitted before half-1 reads them.
                drain_fillers(n_a_units)
                n_ready = (half * HW) // 128
                fillers.extend(
                    [emit_c_tile] * (n_ready - c_state["m"]))
            for h in range(HL):
                if half == 0 and h >= 2:
                    # later head pairs need their q/k m-tiles emitted first
                    drain_fillers(n_v01 + 6 * n_pre)
                window(h, half, drip=2 if half == 0 else 1)
        while norm_pending:
            norm_pending.pop(0)()
        drain_fillers(n_a_units)
        while fillers:
            fillers.pop(0)()
        while c_state["m"] < n_tt:
            emit_c_tile()
        bctx.close()

    nc.compile()
    return nc


_CACHED = {}


def _get_program():
    if "nc" not in _CACHED:
        _CACHED["nc"] = build_program()
    return _CACHED["nc"]


def _bv_cols(bv_local):
    """[FL] head-major bias -> [128, HL//2] per-partition columns matching
    the yT layout (head h -> column h//2, rows 64*(h%2)..+64)."""
    arr = np.zeros((128, HL // 2), dtype=np.float32)
    for h in range(HL):
        arr[64 * (h % 2):64 * (h % 2) + 64, h // 2] = bv_local[h * D:(h + 1) * D]
    return arr


def _pack_w128(w):
    """[C, 128] weight slice -> [128, KT*128] with w[k*128+p, f] at
    [p, k*128+f] (lhsT tiles contiguous per k)."""
    return np.ascontiguousarray(
        w.reshape(KT, 128, -1).transpose(1, 0, 2).reshape(128, -1))


def prepare_in_maps(x, w_qkv, b_qkv, w_out):
    qknp = F8NP if QK_FP8 else BF
    in_maps = []
    for core in range(N_CORES):
        b = core // 2
        g = core % 2
        qs, ks, vs = g * FL, C + g * FL, 2 * C + g * FL
        wq = w_qkv[:, qs:qs + FL]
        wk = w_qkv[:, ks:ks + FL]
        wv_ = w_qkv[:, vs:vs + FL]
        wqk8 = np.stack(
            [_pack_w128(wq[:, m * 128:(m + 1) * 128]) for m in range(4)]
            + [_pack_w128(wk[:, m * 128:(m + 1) * 128]) for m in range(4)],
            axis=1)
        bqk8 = np.stack(
            [b_qkv[qs + m * 128:qs + (m + 1) * 128] for m in range(4)]
            + [b_qkv[ks + m * 128:ks + (m + 1) * 128] for m in range(4)],
            axis=1)
        xb = np.ascontiguousarray(x[b].T)
        in_maps.append({
            "xT8": xb.astype(qknp),
            "xTb": xb.astype(BF),
            "wqk": (wqk8 * W_SCALE).astype(qknp),
            "wv": _pack_w128(wv_).astype(BF),
            "wout": np.ascontiguousarray(
                w_out[g * FL:(g + 1) * FL, :]).astype(BF),
            "bqk": np.ascontiguousarray(bqk8 * W_SCALE, dtype=np.float32),
            "bv": _bv_cols(b_qkv[vs:vs + FL]),
            "trineg": (np.tril(np.ones((128, 128), dtype=np.float32), -1)
                       * np.float32(-1e9)),
        })
    return in_maps


def gather(results, b_out):
    out = np.empty((B, T, C), dtype=np.float32)
    for b in range(B):
        out[b] = (results[2 * b]["out"].astype(np.float32)
                  + results[2 * b + 1]["out"].astype(np.float32) + b_out)
    return out


def kernel(x, w_qkv, b_qkv, w_out, b_out):
    x = np.asarray(x, dtype=np.float32)
    w_qkv = np.asarray(w_qkv, dtype=np.float32)
    b_qkv = np.asarray(b_qkv, dtype=np.float32)
    w_out = np.asarray(w_out, dtype=np.float32)
    b_out = np.asarray(b_out, dtype=np.float32)

    nc = _get_program()
    in_maps = prepare_in_maps(x, w_qkv, b_qkv, w_out)
    res = run_bass_kernel_spmd(nc, in_maps, core_ids=list(range(N_CORES)))
    return gather(res.results, b_out)


if __name__ == "__main__":
    rng = np.random.default_rng(0)
    inputs = {
        "x": rng.standard_normal((B, T, C), dtype=np.float32),
        "w_qkv": rng.standard_normal((C, 3 * C), dtype=np.float32) * 0.02,
        "b_qkv": np.zeros((3 * C,), dtype=np.float32),
        "w_out": rng.standard_normal((C, C), dtype=np.float32) * 0.02,
        "b_out": np.zeros((C,), dtype=np.float32),
    }
    y = kernel(**inputs)
    print("ok", y.shape, y.dtype)


# revision 30
# speedup vs baseline: 1.0236x; 1.0236x over previous
"""Causal self-attention Trainium2 kernel (v6).

Full-model shapes: x [4, 2048, 1024], w_qkv [1024, 3072], b_qkv [3072],
w_out [1024, 1024], b_out [1024].  H=16 heads, D=64.

Sharding: 8 cores = 4 batches x 2 head-groups (tensor parallel).  Each core
computes qkv projection for its 8 heads of its batch, causal attention, and
the partial out-projection (512 of 1024 contraction rows).  The two partials
per batch are summed on the host (the "all-reduce" after out_proj), plus
b_out.

Evolution (traced on HW):
  v2 ~642us: 64 ACT_TABLE_LOADs (exp<->ln set thrash, 82us of ScalarE) +
     4-8us PE gaps at every (head, window) boundary kept PE_HAM throttled
     at 1.2 GHz for 413us (matmuls avg 436ns vs 216ns warm).
  v3 ~529us: activation-table map filtered to natural_log_exp_and_others
     (exp AND ln -> ONE table load); emission restructured for PE density
     (dependency-free qkv/out-proj m-tiles as filler between the
     exp-dependent PVs); 1/Z chain decoupled from PSUM via an SBUF copy.
  v4 ~336us: the v3 trace showed GpSimd UNLOAD_LIB/LOAD_LIB pairs (~6us)
     between partition_broadcast and the diag-mask tensor_mul every window
     -> all diagonal masks moved to the DVE additive path; GpSimd now runs
     ONLY partition_broadcast, one library, no swaps.  HAM stays warm
     (throttle 413us -> 25us).
  v5: q/k/v projections in fp8 DoubleRow: 315us but rel_err 3.5e-2 — numpy
     decomposition shows the v path dominates (y_t for small t is v_t with
     no softmax averaging; max-abs metric is set by those rows), q/k-only
     fp8 is 1.4e-2, out-proj fp8 3.6e-2.
  v6: v path and out-proj back to bf16; q/k stay fp8 DoubleRow (softmax
     soaks their quantization: errors cancel where few keys participate).
     x is streamed in both fp8 (q/k moving operand) and bf16 (v stationary
     operand).  Prefix computes only heads 0/1's q,k so the first window's
     exp starts ~25us earlier; everything else becomes need-scheduled
     filler dripped between s-blocks.

Layout per core:
  - xT [C, T] host-transposed, streamed as [128, KT, chunk] tiles (fp8 for
    q/k, bf16 for v).
  - qT [feat, T] (feature-on-partition, 2 heads per 128-tile), kz per-head
    K-padded [128, T] (64 rows k_h + 64 zero rows -> S matmuls contract over
    full 128 partitions).  q,k carry a x64 factor (fp8 weight scaling);
    the exp scale divides it back out.
  - v natural [T, feat] with a ones column per head so PV yields y_un and
    the softmax denominator Z in one PSUM accumulation.
  - scores S^T [s, t] per (s-block, t-half window); diagonal blocks masked
    pre-exp additive on the DVE.
"""

import sys
from contextlib import ExitStack

import numpy as np

sys.path.insert(0, "/opt/trn_rl_repo")

import ml_dtypes

import concourse.bacc as bacc
import concourse.bass as bass
import concourse.tile as tile
from concourse import mybir
from concourse.bass_utils import run_bass_kernel_spmd

F32 = mybir.dt.float32
BF16 = mybir.dt.bfloat16
BF = ml_dtypes.bfloat16
F8 = mybir.dt.float8e4
F8NP = ml_dtypes.float8_e4m3
DR = mybir.MatmulPerfMode.DoubleRow

B, T, C, H = 4, 2048, 1024, 16
D = C // H  # 64
N_CORES = 8
HL = H // 2      # heads per core = 8
FL = HL * D      # local features = 512
KT = C // 128    # 8 contraction tiles


# Pin every activation to the one table set that holds both exp and ln so
# the table-load fixpoint pass emits a single ACT_TABLE_LOAD.  v2 measured
# 64 loads (82us of ScalarE blockage + the PE stalls they cascade into).
_PIN_SET = "natural_log_exp_and_others"
_real_get_tables = bacc.get_activation_tables


def _pinned_get_tables(arch):
    real = _real_get_tables(arch)
    return {name: (fns if name == _PIN_SET else set())
            for name, fns in real.items()}


bacc.get_activation_tables = _pinned_get_tables


QK_FP8 = True         # q/k projection in fp8 DoubleRow (v, out-proj: bf16)
W_SCALE = 64.0        # fp8 q/k weights are scaled x64 into e4m3's band
EXP_W = 1024          # exp window width (1024 = cross-bank ACT reads)
LAG = 6               # deferred PV chunks: hides exp latency from the PE


def _chunks(t0, tend, grid=512):
    """Aligned chunks [c0, c0+w) covering [t0, tend), clipped to the global
    `grid` so no chunk crosses a grid (= PSUM bank) boundary."""
    out = []
    while t0 < tend:
        w = min(grid - (t0 % grid), tend - t0)
        out.append((t0, w))
        t0 += w
    return out


def build_program(t_len=T):
    nc = bacc.Bacc(None, target_bir_lowering=False, debug=False)
    TT = t_len
    n_tt = TT // 128
    QK_DT, QKNP = (F8, F8NP) if QK_FP8 else (BF16, BF)

    xT8 = nc.declare_dram_parameter("xT8", [C, TT], QK_DT, isOutput=False)
    xTb = nc.declare_dram_parameter("xTb", [C, TT], BF16, isOutput=False)
    # host-packed: wqk[p, m, k*128+f] = w_m[k*128+p, f]; m 0-3 q, 4-7 k tiles
    wqk = nc.declare_dram_parameter("wqk", [128, 8, KT * 128], QK_DT,
                                    isOutput=False)
    wv = nc.declare_dram_parameter("wv", [128, KT * FL], BF16, isOutput=False)
    wout = nc.declare_dram_parameter("wout", [FL, C], BF16, isOutput=False)
    bqk = nc.declare_dram_parameter("bqk", [128, 8], F32, isOutput=False)
    trineg = nc.declare_dram_parameter("trineg", [128, 128], F32,
                                       isOutput=False)
    out = nc.declare_dram_parameter("out", [TT, C], F32, isOutput=True)

    HW = min(1024, TT)   # t-half width for exp windows / S psum tiles
    QW = min(512, TT)    # y accumulation quarter width
    n_half = TT // HW

    with tile.TileContext(nc) as tc, ExitStack() as top:
        persist = top.enter_context(tc.tile_pool(name="persist", bufs=1))
        stream = top.enter_context(tc.tile_pool(name="stream", bufs=3))
        upool = top.enter_context(tc.tile_pool(name="u", bufs=LAG + 2))
        zpool = top.enter_context(tc.tile_pool(name="z", bufs=2))
        ycpool = top.enter_context(tc.tile_pool(name="yc", bufs=4))
        obpool = top.enter_context(tc.tile_pool(name="ob", bufs=3))

        wqk_sb = [persist.tile([128, KT * 128], QK_DT, tag=f"wqk{m}",
                               name=f"wqk{m}") for m in range(8)]
        wv_sb = persist.tile([128, KT * FL], BF16, tag="wv", name="wv_sb")
        qT_sb = [persist.tile([128, TT], BF16, tag=f"qT{j}", name=f"qT{j}")
                 for j in range(4)]
        kz_sb = [persist.tile([128, TT], BF16, tag=f"kz{h}", name=f"kz{h}")
                 for h in range(HL)]
        v_sb = persist.tile([128, n_tt, HL, D + 1], BF16, tag="v", name="v_sb")
        yT_sb = [persist.tile([128, TT], BF16, tag=f"yT{j}", name=f"yT{j}")
                 for j in range(4)]
        wout_sb = [persist.tile([128, C], BF16, tag=f"wo{j}", name=f"wo{j}")
                   for j in range(4)]
        bqk_sb = persist.tile([128, 8], F32, tag="bqk", name="bqk_sb")
        trineg_sb = persist.tile([128, 128], F32, tag="trineg",
                                 name="trineg_sb")

        # -------- qkv projection over t-chunks; x streamed once -----------
        if TT >= 1024:
            achunks = [(0, 256), (256, 256)] + [
                (c, 512) for c in range(512, TT, 512)]
        else:
            achunks = [(c, 256) for c in range(0, TT, 256)]
        n_pre = len([c for c, _ in achunks if c < HW])  # chunks with t < HW
        xtiles8, xtilesb = {}, {}

        def load_chunk(ci, what="8b"):
            c0, ach = achunks[ci]
            if "8" in what:  # one 3D DMA per dtype: ~400ns fixed cost each
                xt8 = stream.tile([128, KT, 512], QK_DT, tag="x8",
                                  name=f"x8_{ci}")
                nc.sync.dma_start(
                    out=xt8[:, :, :ach],
                    in_=xT8.rearrange("(k p) t -> p k t", p=128)[:, :,
                                                                c0:c0 + ach])
                xtiles8[ci] = xt8
            if "b" in what:
                xtb = stream.tile([128, KT, 512], BF16, tag="xb",
                                  name=f"xb_{ci}")
                nc.sync.dma_start(
                    out=xtb[:, :, :ach],
                    in_=xTb.rearrange("(k p) t -> p k t", p=128)[:, :,
                                                                c0:c0 + ach])
                xtilesb[ci] = xtb

        def memset_kz(h):
            zo = 64 * ((h + 1) % 2)
            nc.vector.memset(kz_sb[h][zo:zo + 64, :], 0.0)

        # DMA order = first-use order: the prefix (q/k of heads 0/1, all
        # t<HW chunks) unblocks first, v/bias/weights arrive behind it.
        nc.sync.dma_start(out=wqk_sb[0], in_=wqk[:, 0, :])
        load_chunk(0, "8")
        nc.sync.dma_start(out=wqk_sb[4], in_=wqk[:, 4, :])
        for ci in range(1, n_pre):
            load_chunk(ci, "8")
        nc.sync.dma_start(out=bqk_sb, in_=bqk[:])
        memset_kz(0)
        memset_kz(1)
        nc.vector.memset(v_sb[:, :, :, D], 1.0)  # PV ones column -> Z
        nc.sync.dma_start(out=trineg_sb, in_=trineg[:])
        for ci in range(n_pre):
            load_chunk(ci, "b")
        nc.sync.dma_start(out=wv_sb, in_=wv[:])
        for m in (1, 5, 2, 6, 3, 7):
            nc.sync.dma_start(out=wqk_sb[m], in_=wqk[:, m, :])

        def emit_qk_tile(pool, ci, m):
            c0, ach = achunks[ci]
            xt = xtiles8[ci]
            ps = pool.tile([128, 512], F32, tag="mm", name="aps")
            if QK_FP8:  # DoubleRow: K=256 per matmul
                wm = wqk_sb[m].rearrange("p (kp i f) -> p kp i f", i=2, f=128)
                for kp in range(KT // 2):
                    nc.tensor.matmul(
                        ps[:, :ach],
                        wm[:, kp],
                        xt[:, 2 * kp:2 * kp + 2, :ach],
                        start=(kp == 0), stop=(kp == KT // 2 - 1),
                        perf_mode=DR)
            else:
                for k in range(KT):
                    nc.tensor.matmul(
                        ps[:, :ach],
                        wqk_sb[m][:, k * 128:(k + 1) * 128],
                        xt[:, k, :ach],
                        start=(k == 0), stop=(k == KT - 1))
            bias = bqk_sb[:, m:m + 1]
            if m < 4:
                nc.vector.tensor_scalar_add(
                    qT_sb[m][:, c0:c0 + ach], ps[:, :ach], bias)
            else:
                # split k across the two per-head K-padded tiles
                for par in range(2):
                    nc.vector.tensor_scalar_add(
                        kz_sb[(m - 4) * 2 + par][64 * par:64 * par + 64,
                                                 c0:c0 + ach],
                        ps[64 * par:64 * par + 64, :ach],
                        bias[64 * par:64 * par + 64, :])

        def emit_v_tile(pool, ci, sub):
            c0, ach = achunks[ci]
            xt = xtilesb[ci]
            ps = pool.tile([128, 512], F32, tag="mm", name="aps")
            for k in range(KT):
                nc.tensor.matmul(
                    ps[:, :FL],
                    xt[:, k, sub * 128:(sub + 1) * 128],
                    wv_sb[:, k * FL:(k + 1) * FL],
                    start=(k == 0), stop=(k == KT - 1))
            it = c0 // 128 + sub
            nc.vector.tensor_copy(
                out=v_sb[:, it, :, 0:D],
                in_=ps[:, :FL].rearrange("p (h d) -> p h d", h=HL))

        # prefix: ONLY heads 0/1's q,k over the t<HW chunks, so the first
        # attention window (and with it the ScalarE exp stream) starts as
        # early as possible.  Everything else drips in as filler.
        actx = ExitStack()
        a_pool = actx.enter_context(
            tc.tile_pool(name="a_psum", bufs=4, space="PSUM"))
        for ci in range(n_pre):
            emit_qk_tile(a_pool, ci, 0)
            emit_qk_tile(a_pool, ci, 4)
        actx.close()
        for j in range(4):  # w_out arrives during early attention
            nc.sync.dma_start(out=wout_sb[j], in_=wout[j * 128:(j + 1) * 128, :])

        # -------- attention; remaining qkv + out-proj interleaved ---------
        bctx = ExitStack()
        s_pool = bctx.enter_context(
            tc.tile_pool(name="s_psum", bufs=2, space="PSUM"))
        y_pool = bctx.enter_context(
            tc.tile_pool(name="y_psum", bufs=2, space="PSUM"))
        aux_pool = bctx.enter_context(
            tc.tile_pool(name="aux_psum", bufs=2, space="PSUM"))

        # filler units: dependency-free PE work threaded between the
        # exp-dependent PV matmuls to keep the PE queue dense (HAM warm).
        # Order: v for t<HW (feeds the first windows' PVs), remaining q/k
        # head pairs, then the t>=HW chunks, then out-proj m-tiles.
        fillers = []
        n_consumed = {"n": 0}
        for ci in range(n_pre):
            for sub in range(achunks[ci][1] // 128):
                fillers.append(
                    lambda ci=ci, sub=sub: emit_v_tile(aux_pool, ci, sub))
        n_v01 = len(fillers)
        for ci in range(n_pre):  # chunk-major: frees x stream slots fastest
            for mq in (1, 2, 3):
                fillers.append(lambda ci=ci, m=mq: emit_qk_tile(aux_pool, ci, m))
                fillers.append(
                    lambda ci=ci, m=mq + 4: emit_qk_tile(aux_pool, ci, m))
        for ci in range(n_pre, len(achunks)):
            load_chunk(ci)
            for m in range(8):
                fillers.append(lambda ci=ci, m=m: emit_qk_tile(aux_pool, ci, m))
            for sub in range(achunks[ci][1] // 128):
                fillers.append(
                    lambda ci=ci, sub=sub: emit_v_tile(aux_pool, ci, sub))
        n_a_units = len(fillers) + n_consumed["n"]

        def drain_fillers(n):
            """Emit filler units until at least n have been consumed."""
            while n_consumed["n"] < n and fillers:
                fillers.pop(0)()
                n_consumed["n"] += 1

        c_state = {"m": 0}

        def emit_c_tile():
            m = c_state["m"]
            c_state["m"] += 1
            ob = obpool.tile([128, C], F32, tag="ob", name="ob")
            for n in range(C // 512):
                ps = aux_pool.tile([128, 512], F32, tag="mm", name="cps")
                for j in range(4):
                    nc.tensor.matmul(
                        ps,
                        yT_sb[j][:, m * 128:(m + 1) * 128],
                        wout_sb[j][:, n * 512:(n + 1) * 512],
                        start=(j == 0), stop=(j == 3))
                nc.vector.tensor_copy(out=ob[:, n * 512:(n + 1) * 512], in_=ps)
            nc.sync.dma_start(out=out[m * 128:(m + 1) * 128, :], in_=ob)

        # deferred DVE normalize ops (window w's y*(1/Z)+b), drained at the
        # top of window w+1 so a mul waiting on the GpSimd broadcast never
        # blocks the strict-FIFO DVE queue in front of w+1's mask adds.
        norm_pending = []

        def window(h, half, drip):
            jt = h // 2           # q tile / yT tile index
            po = 64 * (h % 2)     # partition offset within the tile pair
            h0 = half * HW
            while norm_pending:
                norm_pending.pop(0)()
            yq = [y_pool.tile([D + 1, QW], F32, tag="y", name=f"yq{q}")
                  for q in range(HW // QW)]
            pend = []

            def emit_pv(s, c0, w, u):
                q = (c0 - h0) // QW
                nc.tensor.matmul(
                    yq[q][:, (c0 - h0) % QW:(c0 - h0) % QW + w],
                    v_sb[:, s, h, :],
                    u[:, c0 - h0:c0 - h0 + w],
                    start=(s == 0), stop=False,
                    skip_group_check=True)

            for s in range(n_tt):
                if s * 128 >= h0 + HW:
                    break
                t0 = max(s * 128, h0)
                loc0 = t0 - h0
                sps = s_pool.tile([128, HW], F32, tag="s", name="sps")
                chunk_list = _chunks(t0, h0 + HW)
                for (c0, w) in chunk_list:
                    nc.tensor.matmul(
                        sps[:, c0 - h0:c0 - h0 + w],
                        kz_sb[h][:, s * 128:(s + 1) * 128],
                        qT_sb[jt][:, c0:c0 + w],
                        start=True, stop=True,
                        skip_group_check=True)
                if s * 128 >= h0:  # pre-exp additive diag mask (DVE)
                    nc.vector.tensor_add(
                        sps[:, loc0:loc0 + 128], sps[:, loc0:loc0 + 128],
                        trineg_sb)
                u = upool.tile([128, HW], BF16, tag="u", name="u")
                for w0 in range(loc0 // EXP_W * EXP_W, HW, EXP_W):
                    a0, a1 = max(loc0, w0), min(w0 + EXP_W, HW)
                    nc.scalar.activation(
                        out=u[:, a0:a1], in_=sps[:, a0:a1],
                        func=mybir.ActivationFunctionType.Exp,
                        scale=1.0 / (np.sqrt(D) * W_SCALE * W_SCALE))
                for (c0, w) in chunk_list:
                    pend.append((s, c0, w, u))
                while len(pend) > LAG:
                    emit_pv(*pend.pop(0))
                if drip and (drip > 1 or s % 2 == 1):
                    drain_fillers(n_consumed["n"] + drip)
            for p in pend:
                emit_pv(*p)

            # deferred softmax normalization, decoupled from PSUM: one DVE
            # copy to SBUF bf16 frees the yq bank; 1/Z = exp(-ln Z) on the
            # Scalar engine (single pinned table set), GpSimd broadcast
            # from partition 0, bf16 DVE mul+bias into yT (deferred).
            for q in range(HW // QW):
                col0 = h0 + q * QW
                yc = ycpool.tile([D + 1, QW], BF16, tag="yc", name="yc")
                nc.vector.tensor_copy(out=yc, in_=yq[q])
                lnz = zpool.tile([1, QW], F32, tag="lnz", name="lnz")
                nc.scalar.activation(out=lnz, in_=yc[D:D + 1, :],
                                     func=mybir.ActivationFunctionType.Ln)
                rz = zpool.tile([1, QW], BF16, tag="rz", name="rz")
                nc.scalar.activation(out=rz, in_=lnz, scale=-1.0,
                                     func=mybir.ActivationFunctionType.Exp)
                rzb = zpool.tile([64, QW], BF16, tag="rzb", name="rzb")
                nc.gpsimd.partition_broadcast(rzb, rz)

                def norm(jt=jt, po=po, col0=col0, yc=yc, rzb=rzb):
                    # v-bias is folded into b_out on the host (softmax rows
                    # sum to 1, so y = A(x@wv) + bv exactly)
                    dst = yT_sb[jt][po:po + 64, col0:col0 + QW]
                    nc.vector.tensor_mul(dst, yc[0:D, :], rzb)
                norm_pending.append(norm)
            if half == 0 and h + 2 < HL:
                # drip the next head-pair's kz zero-half here instead of
                # upfront: 8x 1.76us DVE memsets at the head of the FIFO
                # blocked the prefix evacs for 6us in the v6 trace
                memset_kz(h + 2)

        for half in range(n_half):
            if half == n_half - 1:
                # out-proj m-tiles of completed halves become the filler;
                # all qkv units must be emitted before half-1 reads them.
                drain_fillers(n_a_units)
                n_ready = (half * HW) // 128
                fillers.extend(
                    [emit_c_tile] * (n_ready - c_state["m"]))
            for h in range(HL):
                if half == 0 and h >= 2:
                    # later head pairs need their q/k m-tiles emitted first
                    drain_fillers(n_v01 + 6 * n_pre)
                window(h, half, drip=2 if half == 0 else 1)
        while norm_pending:
            norm_pending.pop(0)()
        drain_fillers(n_a_units)
        while fillers:
            fillers.pop(0)()
        while c_state["m"] < n_tt:
            emit_c_tile()
        bctx.close()

    nc.compile()
    return nc


_CACHED = {}


def _get_program():
    if "nc" not in _CACHED:
        _CACHED["nc"] = build_program()
    return _CACHED["nc"]


def _pack_w128(w):
    """[C, 128] weight slice -> [128, KT*128] with w[k*128+p, f] at
    [p, k*128+f] (lhsT tiles contiguous per k)."""
    return np.ascontiguousarray(
        w.reshape(KT, 128, -1).transpose(1, 0, 2).reshape(128, -1))


def prepare_in_maps(x, w_qkv, b_qkv, w_out):
    qknp = F8NP if QK_FP8 else BF
    in_maps = []
    for core in range(N_CORES):
        b = core // 2
        g = core % 2
        qs, ks, vs = g * FL, C + g * FL, 2 * C + g * FL
        wq = w_qkv[:, qs:qs + FL]
        wk = w_qkv[:, ks:ks + FL]
        wv_ = w_qkv[:, vs:vs + FL]
        wqk8 = np.stack(
            [_pack_w128(wq[:, m * 128:(m + 1) * 128]) for m in range(4)]
            + [_pack_w128(wk[:, m * 128:(m + 1) * 128]) for m in range(4)],
            axis=1)
        bqk8 = np.stack(
            [b_qkv[qs + m * 128:qs + (m + 1) * 128] for m in range(4)]
            + [b_qkv[ks + m * 128:ks + (m + 1) * 128] for m in range(4)],
            axis=1)
        xb = np.ascontiguousarray(x[b].T)
        in_maps.append({
            "xT8": xb.astype(qknp),
            "xTb": xb.astype(BF),
            "wqk": (wqk8 * W_SCALE).astype(qknp),
            "wv": _pack_w128(wv_).astype(BF),
            "wout": np.ascontiguousarray(
                w_out[g * FL:(g + 1) * FL, :]).astype(BF),
            "bqk": np.ascontiguousarray(bqk8 * W_SCALE, dtype=np.float32),
            "trineg": (np.tril(np.ones((128, 128), dtype=np.float32), -1)
                       * np.float32(-1e9)),
        })
    return in_maps


def gather(results, b_out, b_qkv, w_out):
    # v-bias folded through the out-projection: y = A(x@wv) + bv exactly
    # (softmax rows sum to 1), so out += bv @ w_out once, on the host.
    bias = b_out + b_qkv[2 * C:] @ w_out
    out = np.empty((B, T, C), dtype=np.float32)
    for b in range(B):
        out[b] = (results[2 * b]["out"].astype(np.float32)
                  + results[2 * b + 1]["out"].astype(np.float32) + bias)
    return out


def kernel(x, w_qkv, b_qkv, w_out, b_out):
    x = np.asarray(x, dtype=np.float32)
    w_qkv = np.asarray(w_qkv, dtype=np.float32)
    b_qkv = np.asarray(b_qkv, dtype=np.float32)
    w_out = np.asarray(w_out, dtype=np.float32)
    b_out = np.asarray(b_out, dtype=np.float32)

    nc = _get_program()
    in_maps = prepare_in_maps(x, w_qkv, b_qkv, w_out)
    res = run_bass_kernel_spmd(nc, in_maps, core_ids=list(range(N_CORES)))
    return gather(res.results, b_out, b_qkv, w_out)


if __name__ == "__main__":
    rng = np.random.default_rng(0)
    inputs = {
        "x": rng.standard_normal((B, T, C), dtype=np.float32),
        "w_qkv": rng.standard_normal((C, 3 * C), dtype=np.float32) * 0.02,
        "b_qkv": np.zeros((3 * C,), dtype=np.float32),
        "w_out": rng.standard_normal((C, C), dtype=np.float32) * 0.02,
        "b_out": np.zeros((C,), dtype=np.float32),
    }
    y = kernel(**inputs)
    print("ok", y.shape, y.dtype)
